# revision 2
# baseline (speedup 1.0000x reference)
"""Self-contained TRN2 Bass kernel for nn_MultiHeadAttention_77833397338481 (v2).

kernel(**inputs) takes FULL unsharded inputs (Q/K/V [2,1024,1024], weights
[1024,1024], biases [1024]) and returns the FULL output [2,1024,1024].

Sharding: 8 cores = batch(2) x head-group(4); each group = 16 heads of dim 16
(reference packs heads as d_head=64 "head axis" with per-head dim 16 -> 64
heads of dim 16 total; 16 per core).

Per-core schedule (engine-balanced):
 - fp16 everywhere on-device (psum fp32), packed dense QK projections with
   DMA repack into 32-row head slots, V projection interleaved under
   attention.
 - attention per (n, t, i): 4 row-tiled score matmuls -> exp split between
   the Scalar engine (exact, table exp) and the Vector engine (Schraudolph
   exp2 bit-trick via fp32->int16 convert + f16 bitcast) -> 4 col-tiled
   ctx matmuls accumulating ctx^T + ones-row denominators in one psum bank.
 - softmax normalization: stage copy, DMA den row-gather, reciprocal on DVE,
   PE broadcast, one scalar_tensor_tensor per (n, t).
 - output projection interleaved: sq-blocks 0-3 under the n=1 attention
   stream, 4-7 as tail; f16 partial outputs summed on host with bo.
"""

import numpy as np

import concourse.bacc as bacc
import concourse.mybir as mybir
import concourse.tile as tile

F32 = mybir.dt.float32
F16 = mybir.dt.float16
I16 = mybir.dt.int16
AF = mybir.ActivationFunctionType
ALU = mybir.AluOpType

D = 1024
S = 1024
B = 2
E = 16          # per-head dim
HPC = 16        # heads per core
NQUAD = 4       # head quads (t)
NHQ = 4         # heads per quad (j)
VW = 17 * HPC   # 272
SCALE = 1.0 / 32.0
ND = D // 128
NS = S // 128

# Schraudolph exp2 in fp16 bit domain: exp(x*SCALE) ~ bitcast_f16(int16(x*A + B))
LOG2E = 1.4426950408889634
EXP_A = (1 << 10) * LOG2E * SCALE
EXP_B = 15.0 * 1024 - 44.75

# rounds where the Scalar engine takes both exp pairs (DVE load shedding)
ACT_BOTH = {(0, 3), (2, 3), (1, 7)}  # (t, i); applied for both n


def build_nc():
    nc = bacc.Bacc("TRN2", target_bir_lowering=False, debug=False, num_devices=8)

    xq_d = nc.dram_tensor("xq", [D, S], F16, kind="ExternalInput")
    xk_d = nc.dram_tensor("xk", [D, S], F16, kind="ExternalInput")
    xv_d = nc.dram_tensor("xv", [D, S], F16, kind="ExternalInput")
    wq_d = nc.dram_tensor("wq", [D, 256], F16, kind="ExternalInput")
    wk_d = nc.dram_tensor("wk", [D, 256], F16, kind="ExternalInput")
    wv_d = nc.dram_tensor("wv", [D, VW], F16, kind="ExternalInput")
    wo_d = nc.dram_tensor("wo", [512, D], F16, kind="ExternalInput")
    ind_d = nc.dram_tensor("ind", [NHQ, 128], F16, kind="ExternalInput")
    bvrow_d = nc.dram_tensor("bvrow", [1, VW], F32, kind="ExternalInput")
    bqp_d = nc.dram_tensor("bqp", [128, 2], F32, kind="ExternalInput")
    bkp_d = nc.dram_tensor("bkp", [128, 2], F32, kind="ExternalInput")
    out_d = nc.dram_tensor("out_part", [S, D], F16, kind="ExternalOutput")

    with tile.TileContext(nc) as tc:
        with (
            tc.tile_pool(name="persist", bufs=1) as pp,
            tc.tile_pool(name="work", bufs=1) as wp,
            tc.tile_pool(name="psum", space="PSUM", bufs=1) as ps,
        ):
            # --- exp table warm-up ASAP ---
            dummy = pp.tile([1, 8], F32, name="dummy")
            nc.vector.memset(dummy, 0.0)
            dummy2 = pp.tile([1, 8], F32, name="dummy2")
            nc.scalar.activation(dummy2, dummy, AF.Exp)

            # --- constants ---
            ind_sb = pp.tile([NHQ, 128], F16, name="ind_sb")
            nc.sync.dma_start(out=ind_sb, in_=ind_d[:])
            bvrow_sb = pp.tile([1, VW], F32, name="bvrow_sb")
            nc.sync.dma_start(out=bvrow_sb, in_=bvrow_d[:])
            bq_sb = pp.tile([128, 2], F32, name="bq_sb")
            nc.sync.dma_start(out=bq_sb, in_=bqp_d[:])
            bk_sb = pp.tile([128, 2], F32, name="bk_sb")
            nc.sync.dma_start(out=bk_sb, in_=bkp_d[:])
            ones1 = pp.tile([1, 128], F32, name="ones1")
            nc.vector.memset(ones1, 1.0)
            warm_rhs = pp.tile([1, 512], F16, name="warm_rhs")
            nc.vector.memset(warm_rhs, 0.5)
            ones1h = pp.tile([1, 128], F16, name="ones1h")
            nc.vector.memset(ones1h, 1.0)
            biasB_ps = ps.tile([128, 512], F32, name="biasB_ps", tag="pp", bufs=1)
            nc.tensor.matmul(
                biasB_ps[:, 0:VW], ones1, bvrow_sb, start=True, stop=True
            )
            biasB = pp.tile([128, VW], F32, name="biasB")
            nc.vector.tensor_copy(biasB, biasB_ps[:, 0:VW])

            # --- persistent tiles ---
            qt32 = [pp.tile([128, S], F16, name=f"qt32_{t}") for t in range(NQUAD)]
            kt32 = [pp.tile([128, S], F16, name=f"kt32_{t}") for t in range(NQUAD)]
            va = [pp.tile([128, VW], F16, name=f"va{i}") for i in range(NS)]
            ctxp = [pp.tile([128, S], F16, name=f"ctxp{t}") for t in range(NQUAD)]
            wot_sb = [pp.tile([128, D], F16, name=f"wot{t}") for t in range(NQUAD)]

            # --- weight DMAs (small, first) ---
            wq_sb = [pp.tile([128, 256], F16, name=f"wq{d}") for d in range(ND)]
            wk_sb = [pp.tile([128, 256], F16, name=f"wk{d}") for d in range(ND)]
            for d in range(ND):
                nc.sync.dma_start(out=wq_sb[d], in_=wq_d[128 * d : 128 * (d + 1), :])
            for d in range(ND):
                nc.scalar.dma_start(out=wk_sb[d], in_=wk_d[128 * d : 128 * (d + 1), :])

            # --- xt chunk tiles [128, 512] per (d, nh) ---
            xq_sb = [
                [pp.tile([128, 512], F16, name=f"xq{d}_{h}") for h in range(2)]
                for d in range(ND)
            ]
            xk_sb = [pp.tile([128, 1024], F16, name=f"xk{d}") for d in range(ND)]
            xv_sb = [pp.tile([128, 1024], F16, name=f"xv{d}") for d in range(ND)]

            def dma_xq(nh):
                for d in range(ND):
                    nc.sync.dma_start(
                        out=xq_sb[d][nh],
                        in_=xq_d[128 * d : 128 * (d + 1), 512 * nh : 512 * (nh + 1)],
                    )

            dma_xq(0)
            for d in range(ND):
                nc.scalar.dma_start(out=xk_sb[d], in_=xk_d[128 * d : 128 * (d + 1), :])

            # ---------- projection helpers ----------
            def qk_proj_parts(which, dgrp, nh, nparts=4):
                """Packed-dense projection, emission split into nparts closures."""
                w_sb = wq_sb if which == "q" else wk_sb
                bias = bq_sb if which == "q" else bk_sb
                state = {}

                def part(pi):
                    def go():
                        if pi == 0:
                            state["p"] = ps.tile(
                                [128, 512], F32, name=f"p{which}{dgrp}{nh}",
                                tag="pp", bufs=1,
                            )
                        p = state["p"]
                        dlo = ND * pi // nparts
                        dhi = ND * (pi + 1) // nparts
                        for d in range(dlo, dhi):
                            rhs = (
                                xq_sb[d][nh]
                                if which == "q"
                                else xk_sb[d][:, 512 * nh : 512 * (nh + 1)]
                            )
                            nc.tensor.matmul(
                                p,
                                w_sb[d][:, 128 * dgrp : 128 * (dgrp + 1)],
                                rhs,
                                start=(d == 0),
                                stop=(d == ND - 1),
                            )
                        if pi == nparts - 1:
                            p16 = wp.tile(
                                [128, 512], F16, name=f"p16_{which}{dgrp}{nh}",
                                tag="p16", bufs=2,
                            )
                            nc.vector.tensor_scalar(
                                p16, p, bias[:, dgrp : dgrp + 1], None, ALU.add
                            )
                            dst = qt32 if which == "q" else kt32
                            for mm in range(8):
                                m = 8 * dgrp + mm
                                t, j = m // NHQ, m % NHQ
                                nc.gpsimd.dma_start(
                                    out=dst[t][
                                        32 * j : 32 * j + E,
                                        512 * nh : 512 * (nh + 1),
                                    ],
                                    in_=p16[16 * mm : 16 * mm + E, :],
                                )
                    return go

                return [part(pi) for pi in range(nparts)]

            def qk_proj(which, dgrp, nh):
                for fn in qk_proj_parts(which, dgrp, nh, 1):
                    fn()

            wv_sb = [pp.tile([128, VW], F16, name=f"wv{d}") for d in range(ND)]

            def v_group(i):
                """V projection for sk block i -> va[i] [128, VW]."""
                p = ps.tile([128, 512], F32, name=f"pv{i}", tag="pp", bufs=1)
                for d in range(ND):
                    nc.tensor.matmul(
                        p[:, 0:VW],
                        xv_sb[d][:, 128 * i : 128 * i + 128],
                        wv_sb[d],
                        start=(d == 0),
                        stop=(d == ND - 1),
                    )
                nc.vector.tensor_add(va[i], p[:, 0:VW], biasB)

            # ---------- output projection ----------
            def outproj_group(m, dc):
                po = ps.tile([128, 512], F32, name=f"po{m}{dc}", tag="pp", bufs=1)
                for t in range(NQUAD):
                    nc.tensor.matmul(
                        po,
                        ctxp[t][:, 128 * m : 128 * (m + 1)],
                        wot_sb[t][:, 512 * dc : 512 * (dc + 1)],
                        start=(t == 0),
                        stop=(t == NQUAD - 1),
                    )
                og = wp.tile([128, 512], F16, name=f"og{m}{dc}", tag="og", bufs=3)
                if dc == 0:
                    nc.scalar.activation(og, po, AF.Copy)
                else:
                    nc.vector.tensor_copy(og, po)
                nc.sync.dma_start(
                    out=out_d[128 * m : 128 * (m + 1), 512 * dc : 512 * (dc + 1)],
                    in_=og,
                )

            # ---------- attention ----------
            def attention(n, t, hooks):
                ctx_ps = ps.tile([128, 512], F32, name=f"ctx{n}{t}", tag="ctx", bufs=1)
                for i in range(NS):
                    for fn in hooks.get(i, ()):
                        fn()
                    scA = ps.tile([128, 1024], F32, name=f"scA{n}{t}{i}", tag="sc", bufs=3)
                    scB = ps.tile([128, 1024], F32, name=f"scB{n}{t}{i}", tag="sc", bufs=3)
                    for j in range(NHQ):
                        sch = scA if j < 2 else scB
                        nc.tensor.matmul(
                            sch[:, 512 * (j % 2) : 512 * (j % 2 + 1)],
                            kt32[t][32 * j : 32 * j + E, 128 * i : 128 * (i + 1)],
                            qt32[t][32 * j : 32 * j + E, 512 * n : 512 * (n + 1)],
                            start=True,
                            stop=True,
                            tile_position=(32 * j, 0),
                        )
                    exA = wp.tile([128, 1024], F16, name=f"exA{n}{t}{i}", tag="exf", bufs=4)
                    nc.scalar.activation(exA, scA, AF.Exp, scale=SCALE)
                    if (t, i) in ACT_BOTH:
                        exB = wp.tile(
                            [128, 1024], F16, name=f"exB{n}{t}{i}", tag="exf", bufs=4
                        )
                        nc.scalar.activation(exB, scB, AF.Exp, scale=SCALE)
                        exB_mm = exB
                    else:
                        exBi = wp.tile(
                            [128, 1024], I16, name=f"exB{n}{t}{i}", tag="exi", bufs=3
                        )
                        nc.vector.tensor_scalar(
                            exBi, scB, EXP_A, EXP_B, ALU.mult, ALU.add
                        )
                        exB_mm = exBi.bitcast(F16)
                    for j in range(NHQ):
                        exsrc = exA if j < 2 else exB_mm
                        nc.tensor.matmul(
                            ctx_ps[32 * j : 32 * j + 17, :],
                            va[i][:, 68 * t + 17 * j : 68 * t + 17 * j + 17],
                            exsrc[:, 512 * (j % 2) : 512 * (j % 2 + 1)],
                            start=(i == 0),
                            stop=(i == NS - 1),
                            tile_position=(0, 32 * j),
                        )
                # normalization part 1 (frees the ctx bank quickly)
                with tc.high_priority(offset=-160):
                    stage = wp.tile([128, 512], F32, name=f"st{n}{t}", tag="stage", bufs=2)
                    nc.vector.tensor_copy(stage, ctx_ps)
                    den = wp.tile([NHQ, 512], F32, name=f"den{n}{t}", tag="den", bufs=2)
                    for j in range(NHQ):
                        nc.sync.dma_start(
                            out=den[j : j + 1, :],
                            in_=stage[32 * j + 16 : 32 * j + 17, :],
                        )
                    recip = wp.tile([NHQ, 512], F32, name=f"rc{n}{t}", tag="recip", bufs=2)
                    nc.vector.reciprocal_approx_fast(recip, den)
                    recipr = wp.tile([NHQ, 512], F16, name=f"rr{n}{t}", tag="recipr", bufs=2)
                    nc.vector.tensor_copy(recipr, recip)

                def finish_norm():
                    # part 2 (PE broadcast + stt), deferred off the round boundary
                    with tc.high_priority(offset=-160):
                        rbw = ps.tile([128, 512], F32, name=f"rbw{n}{t}", tag="pp", bufs=1)
                        nc.tensor.matmul(rbw, ind_sb, recipr, start=True, stop=True)
                        nc.vector.scalar_tensor_tensor(
                            ctxp[t][:, 512 * n : 512 * (n + 1)],
                            rbw,
                            1.0,
                            stage,
                            ALU.mult,
                            ALU.mult,
                        )

                return finish_norm

            # ---------- emission schedule ----------
            for w_i in range(12):
                wps = ps.tile([128, 1024], F32, name=f"warm{w_i}", tag="sc", bufs=3)
                nc.tensor.matmul(
                    wps[:, 0:512], ones1h, warm_rhs, start=True, stop=True
                )
            qk_proj("q", 0, 0)
            qk_proj("k", 0, 0)
            qk_proj("k", 0, 1)
            for d in range(ND):
                nc.gpsimd.dma_start(out=wv_sb[d], in_=wv_d[128 * d : 128 * (d + 1), :])
            for d in range(ND):
                nc.gpsimd.dma_start(out=xv_sb[d], in_=xv_d[128 * d : 128 * (d + 1), :])
            dma_xq(1)

            fin = attention(
                0, 0,
                {i: [(lambda i=i: v_group(i))] for i in range(NS)},
            )
            # dense projection block: rest of q/k while PE is warm
            qk_proj("q", 1, 0)
            qk_proj("k", 1, 0)
            qk_proj("k", 1, 1)
            qk_proj("q", 0, 1)
            qk_proj("q", 1, 1)
            fin = attention(0, 1, {2: [fin]})
            fin = attention(
                0, 2,
                {
                    2: [fin],
                    6: [
                        lambda: [
                            nc.sync.dma_start(
                                out=wot_sb[t], in_=wo_d[128 * t : 128 * (t + 1), :]
                            )
                            for t in range(NQUAD)
                        ]
                    ],
                },
            )
            fin = attention(0, 3, {2: [fin]})
            fin = attention(1, 0, {2: [fin]})
            fin = attention(
                1, 1,
                {
                    1: [lambda: outproj_group(0, 0)],
                    2: [fin],
                    3: [lambda: outproj_group(0, 1)],
                    5: [lambda: outproj_group(1, 0)],
                    7: [lambda: outproj_group(1, 1)],
                },
            )
            fin = attention(
                1, 2,
                {
                    1: [lambda: outproj_group(2, 0)],
                    2: [fin],
                    3: [lambda: outproj_group(2, 1)],
                    5: [lambda: outproj_group(3, 0)],
                    7: [lambda: outproj_group(3, 1)],
                },
            )
            fin2 = attention(1, 3, {2: [fin]})
            fin2()
            for m in range(4, 8):
                for dc in range(2):
                    outproj_group(m, dc)

    nc.finalize()
    return nc


# ================= host-side prep =================


def prep_core_weights(g, Wq, bq, Wk, bk, Wv, bv, Wo):
    C0 = 256 * g
    wq = np.ascontiguousarray(Wq[C0 : C0 + 256, :].T).astype(np.float16)
    wk = np.ascontiguousarray(Wk[C0 : C0 + 256, :].T).astype(np.float16)
    wv = np.zeros((D, VW), np.float16)
    bvrow = np.zeros((1, VW), np.float32)
    for m in range(HPC):
        src = C0 + E * m
        wv[:, 17 * m : 17 * m + E] = Wv[src : src + E, :].T.astype(np.float16)
        bvrow[0, 17 * m : 17 * m + E] = bv[src : src + E]
        bvrow[0, 17 * m + E] = 1.0
    wo = np.zeros((512, D), np.float16)
    for t in range(NQUAD):
        for j in range(NHQ):
            src = C0 + E * (NHQ * t + j)
            wo[128 * t + 32 * j : 128 * t + 32 * j + E, :] = Wo[:, src : src + E].T.astype(
                np.float16
            )
    bqp = bq[C0 : C0 + 256].reshape(2, 128).T.astype(np.float32)
    bkp = bk[C0 : C0 + 256].reshape(2, 128).T.astype(np.float32)
    ind = np.zeros((NHQ, 128), np.float16)
    for j in range(NHQ):
        ind[j, 32 * j : 32 * j + E] = 1.0
    return {
        "wq": wq,
        "wk": wk,
        "wv": np.ascontiguousarray(wv),
        "wo": np.ascontiguousarray(wo),
        "bvrow": bvrow,
        "bqp": np.ascontiguousarray(bqp),
        "bkp": np.ascontiguousarray(bkp),
        "ind": ind,
    }


def prep_in_maps(Q, K, V, Wq, bq, Wk, bk, Wv, bv, Wo):
    group_w = [prep_core_weights(g, Wq, bq, Wk, bk, Wv, bv, Wo) for g in range(4)]
    xt = []
    for b in range(B):
        xt.append(
            {
                "xq": np.ascontiguousarray(Q[b].T).astype(np.float16),
                "xk": np.ascontiguousarray(K[b].T).astype(np.float16),
                "xv": np.ascontiguousarray(V[b].T).astype(np.float16),
            }
        )
    in_maps = []
    for c in range(8):
        b, g = c // 4, c % 4
        m = dict(group_w[g])
        m.update(xt[b])
        in_maps.append(m)
    return in_maps


def assemble_output(results, bo):
    out = np.zeros((B, S, D), np.float32)
    for b in range(B):
        acc = np.zeros((S, D), np.float64)
        for g in range(4):
            acc += results[4 * b + g]["out_part"].astype(np.float64)
        out[b] = (acc + bo.astype(np.float64)).astype(np.float32)
    return out


_NC_CACHE = {}


def _get_nc():
    if "nc" not in _NC_CACHE:
        _NC_CACHE["nc"] = build_nc()
    return _NC_CACHE["nc"]


def kernel(Q, K, V, Wq, bq, Wk, bk, Wv, bv, Wo, bo):
    import time

    from concourse.bass_utils import run_bass_kernel_spmd

    nc = _get_nc()
    in_maps = prep_in_maps(
        np.asarray(Q, np.float32),
        np.asarray(K, np.float32),
        np.asarray(V, np.float32),
        np.asarray(Wq, np.float32),
        np.asarray(bq, np.float32),
        np.asarray(Wk, np.float32),
        np.asarray(bk, np.float32),
        np.asarray(Wv, np.float32),
        np.asarray(bv, np.float32),
        np.asarray(Wo, np.float32),
    )
    last = None
    for attempt in range(3):
        try:
            res = run_bass_kernel_spmd(nc, in_maps, list(range(8)))
            return assemble_output(res.results, np.asarray(bo, np.float32))
        except Exception as e:
            last = e
            time.sleep(3)
    raise last


# revision 3
# speedup vs baseline: 1.0191x; 1.0191x over previous
"""Self-contained TRN2 Bass kernel for nn_MultiHeadAttention_77833397338481 (v2).

kernel(**inputs) takes FULL unsharded inputs (Q/K/V [2,1024,1024], weights
[1024,1024], biases [1024]) and returns the FULL output [2,1024,1024].

Sharding: 8 cores = batch(2) x head-group(4); each group = 16 heads of dim 16
(reference packs heads as d_head=64 "head axis" with per-head dim 16 -> 64
heads of dim 16 total; 16 per core).

Per-core schedule (engine-balanced):
 - fp16 everywhere on-device (psum fp32), packed dense QK projections with
   DMA repack into 32-row head slots, V projection interleaved under
   attention.
 - attention per (n, t, i): 4 row-tiled score matmuls -> exp split between
   the Scalar engine (exact, table exp) and the Vector engine (Schraudolph
   exp2 bit-trick via fp32->int16 convert + f16 bitcast) -> 4 col-tiled
   ctx matmuls accumulating ctx^T + ones-row denominators in one psum bank.
 - softmax normalization: stage copy, DMA den row-gather, reciprocal on DVE,
   PE broadcast, one scalar_tensor_tensor per (n, t).
 - output projection interleaved: sq-blocks 0-3 under the n=1 attention
   stream, 4-7 as tail; f16 partial outputs summed on host with bo.
"""

import numpy as np

import concourse.bacc as bacc
import concourse.mybir as mybir
import concourse.tile as tile

F32 = mybir.dt.float32
F16 = mybir.dt.float16
I16 = mybir.dt.int16
AF = mybir.ActivationFunctionType
ALU = mybir.AluOpType

D = 1024
S = 1024
B = 2
E = 16          # per-head dim
HPC = 16        # heads per core
NQUAD = 4       # head quads (t)
NHQ = 4         # heads per quad (j)
VW = 17 * HPC   # 272
SCALE = 1.0 / 32.0
ND = D // 128
NS = S // 128

# Schraudolph exp2 in fp16 bit domain: exp(x*SCALE) ~ bitcast_f16(int16(x*A + B))
LOG2E = 1.4426950408889634
EXP_A = (1 << 10) * LOG2E * SCALE
EXP_B = 15.0 * 1024 - 44.75

# rounds where the Scalar engine takes both exp pairs (DVE load shedding)
ACT_BOTH = {(0, 3), (2, 3), (1, 7)}  # (t, i); applied for both n


def build_nc():
    nc = bacc.Bacc("TRN2", target_bir_lowering=False, debug=False, num_devices=8)

    xq_d = nc.dram_tensor("xq", [D, S], F16, kind="ExternalInput")
    xk_d = nc.dram_tensor("xk", [D, S], F16, kind="ExternalInput")
    xv_d = nc.dram_tensor("xv", [D, S], F16, kind="ExternalInput")
    wq_d = nc.dram_tensor("wq", [D, 256], F16, kind="ExternalInput")
    wk_d = nc.dram_tensor("wk", [D, 256], F16, kind="ExternalInput")
    wv_d = nc.dram_tensor("wv", [D, VW], F16, kind="ExternalInput")
    wo_d = nc.dram_tensor("wo", [512, D], F16, kind="ExternalInput")
    ind_d = nc.dram_tensor("ind", [NHQ, 128], F16, kind="ExternalInput")
    bvrow_d = nc.dram_tensor("bvrow", [1, VW], F32, kind="ExternalInput")
    bqp_d = nc.dram_tensor("bqp", [128, 2], F32, kind="ExternalInput")
    bkp_d = nc.dram_tensor("bkp", [128, 2], F32, kind="ExternalInput")
    out_d = nc.dram_tensor("out_part", [S, D], F16, kind="ExternalOutput")

    with tile.TileContext(nc) as tc:
        with (
            tc.tile_pool(name="persist", bufs=1) as pp,
            tc.tile_pool(name="work", bufs=1) as wp,
            tc.tile_pool(name="psum", space="PSUM", bufs=1) as ps,
        ):
            # --- exp table warm-up ASAP ---
            dummy = pp.tile([1, 8], F32, name="dummy")
            nc.vector.memset(dummy, 0.0)
            dummy2 = pp.tile([1, 8], F32, name="dummy2")
            nc.scalar.activation(dummy2, dummy, AF.Exp)

            # --- constants ---
            ind_sb = pp.tile([NHQ, 128], F16, name="ind_sb")
            nc.sync.dma_start(out=ind_sb, in_=ind_d[:])
            bvrow_sb = pp.tile([1, VW], F32, name="bvrow_sb")
            nc.sync.dma_start(out=bvrow_sb, in_=bvrow_d[:])
            bq_sb = pp.tile([128, 2], F32, name="bq_sb")
            nc.sync.dma_start(out=bq_sb, in_=bqp_d[:])
            bk_sb = pp.tile([128, 2], F32, name="bk_sb")
            nc.sync.dma_start(out=bk_sb, in_=bkp_d[:])
            ones1 = pp.tile([1, 128], F32, name="ones1")
            nc.vector.memset(ones1, 1.0)
            warm_rhs = pp.tile([1, 512], F16, name="warm_rhs")
            nc.vector.memset(warm_rhs, 0.5)
            ones1h = pp.tile([1, 128], F16, name="ones1h")
            nc.vector.memset(ones1h, 1.0)
            biasB_ps = ps.tile([128, 512], F32, name="biasB_ps", tag="pp", bufs=1)
            nc.tensor.matmul(
                biasB_ps[:, 0:VW], ones1, bvrow_sb, start=True, stop=True
            )
            biasB = pp.tile([128, VW], F32, name="biasB")
            nc.vector.tensor_copy(biasB, biasB_ps[:, 0:VW])

            # --- persistent tiles ---
            qt32 = [pp.tile([128, S], F16, name=f"qt32_{t}") for t in range(NQUAD)]
            kt32 = [pp.tile([128, S], F16, name=f"kt32_{t}") for t in range(NQUAD)]
            va = [pp.tile([128, VW], F16, name=f"va{i}") for i in range(NS)]
            ctxp = [pp.tile([128, S], F16, name=f"ctxp{t}") for t in range(NQUAD)]
            wot_sb = [pp.tile([128, D], F16, name=f"wot{t}") for t in range(NQUAD)]

            # --- weight DMAs (small, first) ---
            wq_sb = [pp.tile([128, 256], F16, name=f"wq{d}") for d in range(ND)]
            wk_sb = [pp.tile([128, 256], F16, name=f"wk{d}") for d in range(ND)]
            for d in range(ND):
                nc.sync.dma_start(out=wq_sb[d], in_=wq_d[128 * d : 128 * (d + 1), :])
            for d in range(ND):
                nc.scalar.dma_start(out=wk_sb[d], in_=wk_d[128 * d : 128 * (d + 1), :])

            # --- xt chunk tiles [128, 512] per (d, nh) ---
            xq_sb = [
                [pp.tile([128, 512], F16, name=f"xq{d}_{h}") for h in range(2)]
                for d in range(ND)
            ]
            xk_sb = [pp.tile([128, 1024], F16, name=f"xk{d}") for d in range(ND)]
            xv_sb = [pp.tile([128, 1024], F16, name=f"xv{d}") for d in range(ND)]

            def dma_xq(nh):
                for d in range(ND):
                    nc.sync.dma_start(
                        out=xq_sb[d][nh],
                        in_=xq_d[128 * d : 128 * (d + 1), 512 * nh : 512 * (nh + 1)],
                    )

            dma_xq(0)
            for d in range(ND):
                nc.scalar.dma_start(out=xk_sb[d], in_=xk_d[128 * d : 128 * (d + 1), :])

            # ---------- projection helpers ----------
            def qk_proj_parts(which, dgrp, nh, nparts=4):
                """Packed-dense projection, emission split into nparts closures."""
                w_sb = wq_sb if which == "q" else wk_sb
                bias = bq_sb if which == "q" else bk_sb
                state = {}

                def part(pi):
                    def go():
                        if pi == 0:
                            state["p"] = ps.tile(
                                [128, 512], F32, name=f"p{which}{dgrp}{nh}",
                                tag="pp", bufs=1,
                            )
                        p = state["p"]
                        dlo = ND * pi // nparts
                        dhi = ND * (pi + 1) // nparts
                        for d in range(dlo, dhi):
                            rhs = (
                                xq_sb[d][nh]
                                if which == "q"
                                else xk_sb[d][:, 512 * nh : 512 * (nh + 1)]
                            )
                            nc.tensor.matmul(
                                p,
                                w_sb[d][:, 128 * dgrp : 128 * (dgrp + 1)],
                                rhs,
                                start=(d == 0),
                                stop=(d == ND - 1),
                            )
                        if pi == nparts - 1:
                            p16 = wp.tile(
                                [128, 512], F16, name=f"p16_{which}{dgrp}{nh}",
                                tag="p16", bufs=2,
                            )
                            nc.vector.tensor_scalar(
                                p16, p, bias[:, dgrp : dgrp + 1], None, ALU.add
                            )
                            dst = qt32 if which == "q" else kt32
                            for mm in range(8):
                                m = 8 * dgrp + mm
                                t, j = m // NHQ, m % NHQ
                                nc.gpsimd.dma_start(
                                    out=dst[t][
                                        32 * j : 32 * j + E,
                                        512 * nh : 512 * (nh + 1),
                                    ],
                                    in_=p16[16 * mm : 16 * mm + E, :],
                                )
                    return go

                return [part(pi) for pi in range(nparts)]

            def qk_proj(which, dgrp, nh):
                for fn in qk_proj_parts(which, dgrp, nh, 1):
                    fn()

            wv_sb = [pp.tile([128, VW], F16, name=f"wv{d}") for d in range(ND)]

            def v_group(i):
                """V projection for sk block i -> va[i] [128, VW]."""
                p = ps.tile([128, 512], F32, name=f"pv{i}", tag="pp", bufs=1)
                for d in range(ND):
                    nc.tensor.matmul(
                        p[:, 0:VW],
                        xv_sb[d][:, 128 * i : 128 * i + 128],
                        wv_sb[d],
                        start=(d == 0),
                        stop=(d == ND - 1),
                    )
                nc.vector.tensor_add(va[i], p[:, 0:VW], biasB)

            # ---------- output projection ----------
            def outproj_group(m, dc):
                po = ps.tile([128, 512], F32, name=f"po{m}{dc}", tag="pp", bufs=1)
                for t in range(NQUAD):
                    nc.tensor.matmul(
                        po,
                        ctxp[t][:, 128 * m : 128 * (m + 1)],
                        wot_sb[t][:, 512 * dc : 512 * (dc + 1)],
                        start=(t == 0),
                        stop=(t == NQUAD - 1),
                    )
                og = wp.tile([128, 512], F16, name=f"og{m}{dc}", tag="og", bufs=3)
                if dc == 0:
                    nc.scalar.activation(og, po, AF.Copy)
                else:
                    nc.vector.tensor_copy(og, po)
                nc.sync.dma_start(
                    out=out_d[128 * m : 128 * (m + 1), 512 * dc : 512 * (dc + 1)],
                    in_=og,
                )

            # ---------- attention ----------
            def attention(n, t, hooks):
                ctx_ps = ps.tile([128, 512], F32, name=f"ctx{n}{t}", tag="ctx", bufs=1)
                for i in range(NS):
                    for fn in hooks.get(i, ()):
                        fn()
                    scA = ps.tile([128, 1024], F32, name=f"scA{n}{t}{i}", tag="sc", bufs=3)
                    scB = ps.tile([128, 1024], F32, name=f"scB{n}{t}{i}", tag="sc", bufs=3)
                    for j in range(NHQ):
                        sch = scA if j < 2 else scB
                        nc.tensor.matmul(
                            sch[:, 512 * (j % 2) : 512 * (j % 2 + 1)],
                            kt32[t][32 * j : 32 * j + E, 128 * i : 128 * (i + 1)],
                            qt32[t][32 * j : 32 * j + E, 512 * n : 512 * (n + 1)],
                            start=True,
                            stop=True,
                            tile_position=(32 * j, 0),
                        )
                    exA = wp.tile([128, 1024], F16, name=f"exA{n}{t}{i}", tag="exf", bufs=4)
                    nc.scalar.activation(exA, scA, AF.Exp, scale=SCALE)
                    if (t, i) in ACT_BOTH:
                        exB = wp.tile(
                            [128, 1024], F16, name=f"exB{n}{t}{i}", tag="exf", bufs=4
                        )
                        nc.scalar.activation(exB, scB, AF.Exp, scale=SCALE)
                        exB_mm = exB
                    else:
                        exBi = wp.tile(
                            [128, 1024], I16, name=f"exB{n}{t}{i}", tag="exi", bufs=3
                        )
                        nc.vector.tensor_scalar(
                            exBi, scB, EXP_A, EXP_B, ALU.mult, ALU.add
                        )
                        exB_mm = exBi.bitcast(F16)
                    for j in range(NHQ):
                        exsrc = exA if j < 2 else exB_mm
                        nc.tensor.matmul(
                            ctx_ps[32 * j : 32 * j + 17, :],
                            va[i][:, 68 * t + 17 * j : 68 * t + 17 * j + 17],
                            exsrc[:, 512 * (j % 2) : 512 * (j % 2 + 1)],
                            start=(i == 0),
                            stop=(i == NS - 1),
                            tile_position=(0, 32 * j),
                        )
                # normalization part 1 (frees the ctx bank quickly)
                with tc.high_priority(offset=-160):
                    stage = wp.tile([128, 512], F32, name=f"st{n}{t}", tag="stage", bufs=2)
                    nc.vector.tensor_copy(stage, ctx_ps)
                    den = wp.tile([NHQ, 512], F32, name=f"den{n}{t}", tag="den", bufs=2)
                    for j in range(NHQ):
                        nc.sync.dma_start(
                            out=den[j : j + 1, :],
                            in_=stage[32 * j + 16 : 32 * j + 17, :],
                        )
                    recip = wp.tile([NHQ, 512], F32, name=f"rc{n}{t}", tag="recip", bufs=2)
                    nc.vector.reciprocal_approx_fast(recip, den)
                    recipr = wp.tile([NHQ, 512], F16, name=f"rr{n}{t}", tag="recipr", bufs=2)
                    nc.vector.tensor_copy(recipr, recip)

                def finish_norm():
                    # part 2 (PE broadcast + stt), deferred off the round boundary
                    with tc.high_priority(offset=-160):
                        rbw = ps.tile([128, 512], F32, name=f"rbw{n}{t}", tag="pp", bufs=1)
                        nc.tensor.matmul(rbw, ind_sb, recipr, start=True, stop=True)
                        nc.vector.scalar_tensor_tensor(
                            ctxp[t][:, 512 * n : 512 * (n + 1)],
                            rbw,
                            1.0,
                            stage,
                            ALU.mult,
                            ALU.mult,
                        )

                return finish_norm

            # ---------- emission schedule ----------
            for w_i in range(20):
                wps = ps.tile([128, 1024], F32, name=f"warm{w_i}", tag="sc", bufs=3)
                nc.tensor.matmul(
                    wps[:, 0:512], ones1h, warm_rhs, start=True, stop=True
                )
            qk_proj("q", 0, 0)
            qk_proj("k", 0, 0)
            qk_proj("k", 0, 1)
            for d in range(ND):
                nc.gpsimd.dma_start(out=wv_sb[d], in_=wv_d[128 * d : 128 * (d + 1), :])
            for d in range(ND):
                nc.gpsimd.dma_start(out=xv_sb[d], in_=xv_d[128 * d : 128 * (d + 1), :])
            dma_xq(1)

            fin = attention(
                0, 0,
                {i: [(lambda i=i: v_group(i))] for i in range(NS)},
            )
            # dense projection block: rest of q/k while PE is warm
            qk_proj("q", 1, 0)
            qk_proj("k", 1, 0)
            qk_proj("k", 1, 1)
            qk_proj("q", 0, 1)
            qk_proj("q", 1, 1)
            fin = attention(0, 1, {2: [fin]})
            fin = attention(
                0, 2,
                {
                    2: [fin],
                    6: [
                        lambda: [
                            nc.sync.dma_start(
                                out=wot_sb[t], in_=wo_d[128 * t : 128 * (t + 1), :]
                            )
                            for t in range(NQUAD)
                        ]
                    ],
                },
            )
            fin = attention(0, 3, {2: [fin]})
            fin = attention(1, 0, {2: [fin]})
            fin = attention(
                1, 1,
                {
                    1: [lambda: outproj_group(0, 0)],
                    2: [fin],
                    3: [lambda: outproj_group(0, 1)],
                    5: [lambda: outproj_group(1, 0)],
                    7: [lambda: outproj_group(1, 1)],
                },
            )
            fin = attention(
                1, 2,
                {
                    1: [lambda: outproj_group(2, 0)],
                    2: [fin],
                    3: [lambda: outproj_group(2, 1)],
                    5: [lambda: outproj_group(3, 0)],
                    7: [lambda: outproj_group(3, 1)],
                },
            )
            fin2 = attention(1, 3, {2: [fin]})
            fin2()
            for m in range(4, 8):
                for dc in range(2):
                    outproj_group(m, dc)

    nc.finalize()
    return nc


# ================= host-side prep =================


def prep_core_weights(g, Wq, bq, Wk, bk, Wv, bv, Wo):
    C0 = 256 * g
    wq = np.ascontiguousarray(Wq[C0 : C0 + 256, :].T).astype(np.float16)
    wk = np.ascontiguousarray(Wk[C0 : C0 + 256, :].T).astype(np.float16)
    wv = np.zeros((D, VW), np.float16)
    bvrow = np.zeros((1, VW), np.float32)
    for m in range(HPC):
        src = C0 + E * m
        wv[:, 17 * m : 17 * m + E] = Wv[src : src + E, :].T.astype(np.float16)
        bvrow[0, 17 * m : 17 * m + E] = bv[src : src + E]
        bvrow[0, 17 * m + E] = 1.0
    wo = np.zeros((512, D), np.float16)
    for t in range(NQUAD):
        for j in range(NHQ):
            src = C0 + E * (NHQ * t + j)
            wo[128 * t + 32 * j : 128 * t + 32 * j + E, :] = Wo[:, src : src + E].T.astype(
                np.float16
            )
    bqp = bq[C0 : C0 + 256].reshape(2, 128).T.astype(np.float32)
    bkp = bk[C0 : C0 + 256].reshape(2, 128).T.astype(np.float32)
    ind = np.zeros((NHQ, 128), np.float16)
    for j in range(NHQ):
        ind[j, 32 * j : 32 * j + E] = 1.0
    return {
        "wq": wq,
        "wk": wk,
        "wv": np.ascontiguousarray(wv),
        "wo": np.ascontiguousarray(wo),
        "bvrow": bvrow,
        "bqp": np.ascontiguousarray(bqp),
        "bkp": np.ascontiguousarray(bkp),
        "ind": ind,
    }


def prep_in_maps(Q, K, V, Wq, bq, Wk, bk, Wv, bv, Wo):
    group_w = [prep_core_weights(g, Wq, bq, Wk, bk, Wv, bv, Wo) for g in range(4)]
    xt = []
    for b in range(B):
        xt.append(
            {
                "xq": np.ascontiguousarray(Q[b].T).astype(np.float16),
                "xk": np.ascontiguousarray(K[b].T).astype(np.float16),
                "xv": np.ascontiguousarray(V[b].T).astype(np.float16),
            }
        )
    in_maps = []
    for c in range(8):
        b, g = c // 4, c % 4
        m = dict(group_w[g])
        m.update(xt[b])
        in_maps.append(m)
    return in_maps


def assemble_output(results, bo):
    out = np.zeros((B, S, D), np.float32)
    for b in range(B):
        acc = np.zeros((S, D), np.float64)
        for g in range(4):
            acc += results[4 * b + g]["out_part"].astype(np.float64)
        out[b] = (acc + bo.astype(np.float64)).astype(np.float32)
    return out


_NC_CACHE = {}


def _get_nc():
    if "nc" not in _NC_CACHE:
        _NC_CACHE["nc"] = build_nc()
    return _NC_CACHE["nc"]


def kernel(Q, K, V, Wq, bq, Wk, bk, Wv, bv, Wo, bo):
    import time

    from concourse.bass_utils import run_bass_kernel_spmd

    nc = _get_nc()
    in_maps = prep_in_maps(
        np.asarray(Q, np.float32),
        np.asarray(K, np.float32),
        np.asarray(V, np.float32),
        np.asarray(Wq, np.float32),
        np.asarray(bq, np.float32),
        np.asarray(Wk, np.float32),
        np.asarray(bk, np.float32),
        np.asarray(Wv, np.float32),
        np.asarray(bv, np.float32),
        np.asarray(Wo, np.float32),
    )
    last = None
    for attempt in range(3):
        try:
            res = run_bass_kernel_spmd(nc, in_maps, list(range(8)))
            return assemble_output(res.results, np.asarray(bo, np.float32))
        except Exception as e:
            last = e
            time.sleep(3)
    raise last
